# revision 5
# baseline (speedup 1.0000x reference)
"""Soft decision-tree (MoE-routing style) model on 8 Trainium2 NeuronCores.

Computation (see reference):
    d      = sigmoid(x @ W^T)                  x:[B,1024]  W:[1023,1024]
    probs  = level-by-level path products       -> [B, 1024] leaf probs
    out    = softmax(probs @ L, axis=1)         L:[1024,1024]

Strategy (per core, data-parallel over batch):
  * Everything is kept in a [contraction-on-partitions] layout so no
    transposes are ever needed on device:
      GEMM1: z[slot, b]   = Wp^T-chunks (lhsT) x x^T-chunks (rhs)
      GEMM2: logit[b, o]  = P10-chunks  (lhsT) x L-chunks   (rhs)
  * Host pre-permutes weights:
      - node dim padded 1023 -> 1024 "slots", level l at [2^l, 2^(l+1)),
        ordered little-endian within the level (so the on-device tree
        recursion is pure concat, never interleave).
      - leaf predictions permuted by 10-bit bit-reversal to match.
  * Tree levels 0-6 are evaluated in log-space with a single PE matmul
    against a constant +/-0/1 selection matrix (M7), so every vector-engine
    op in the kernel is full-width and partition-aligned (levels 7-9 are
    plain full-width mul/sub).
  * Matmuls run in float32r (full fp32 precision at 1 col/cycle for
    free-dim >= 256 on TRN2).
"""

import numpy as np

import concourse.bacc as bacc
import concourse.bass as bass
import concourse.mybir as mybir
import concourse.tile as tile
from concourse.bass_utils import run_bass_kernel_spmd

AF = mybir.ActivationFunctionType
f32 = mybir.dt.float32
f32r = mybir.dt.float32r

MAX_DEPTH = 10
B = 32768
F = 1024
NOUT = 1024
NLEAF = 1024
NCORES = 8
BL = B // NCORES          # rows per core
BLOCK = 512               # batch columns processed per block
NBLOCKS = BL // BLOCK


def _bitrev(i: int, bits: int) -> int:
    r = 0
    for b in range(bits):
        r = (r << 1) | ((i >> b) & 1)
    return r


def _round_f32r(a: np.ndarray) -> np.ndarray:
    """Round fp32 to fp32r (1s/8e/11m, value held in the top 20 bits) with RNE."""
    u = np.ascontiguousarray(a, dtype=np.float32).view(np.uint32)
    lo = u & np.uint32(0xFFF)
    base = u & np.uint32(0xFFFFF000)
    rnd = (lo > 0x800) | ((lo == 0x800) & (((u >> np.uint32(12)) & np.uint32(1)) == 1))
    out = base + (rnd.astype(np.uint32) << np.uint32(12))
    return out.view(np.float32)


def _host_prep(feature_thresholds: np.ndarray, leaf_predictions: np.ndarray):
    """Build the permuted/padded constant tensors."""
    ft = np.asarray(feature_thresholds, dtype=np.float32)
    lp = np.asarray(leaf_predictions, dtype=np.float32)

    # Padded node slots: level l occupies [2^l, 2^(l+1)), little-endian order
    # within the level: slot 2^l + j holds BFS node (2^l - 1) + bitrev_l(j).
    wp = np.zeros((1024, F), dtype=np.float32)
    for lvl in range(MAX_DEPTH):
        n = 1 << lvl
        src = np.fromiter(
            ((n - 1) + _bitrev(j, lvl) for j in range(n)), dtype=np.int64, count=n
        )
        wp[n : 2 * n] = ft[src]
    wt = np.ascontiguousarray(wp.T)  # [F, 1024 slots]

    # Leaf predictions in little-endian leaf order.
    perm = np.fromiter(
        (_bitrev(i, MAX_DEPTH) for i in range(NLEAF)), dtype=np.int64, count=NLEAF
    )
    lperm = np.ascontiguousarray(lp[perm])  # [1024, NOUT]

    # M7 selection matrix: logP7[j] = sum over levels 0..6 of ln(d or 1-d).
    # Rows 0..127   multiply ln(sigmoid(z))  of slot s.
    # Rows 128..255 multiply ln(sigmoid(-z)) of slot s-128.
    m7 = np.zeros((256, 128), dtype=np.float32)
    for j in range(128):
        for lvl in range(7):
            slot = (1 << lvl) + (j & ((1 << lvl) - 1))
            bit = (j >> lvl) & 1
            m7[slot + 128 * bit, j] = 1.0
    return _round_f32r(wt), _round_f32r(lperm), m7


def _build_program(n_blocks: int = NBLOCKS, block: int = BLOCK) -> bass.Bass:
    nc = bacc.Bacc()
    nb = n_blocks * block
    xt = nc.dram_tensor("xt", [F, nb], f32r, kind="ExternalInput")
    wt = nc.dram_tensor("wt", [F, 1024], f32r, kind="ExternalInput")
    lp = nc.dram_tensor("lp", [NLEAF, NOUT], f32r, kind="ExternalInput")
    m7 = nc.dram_tensor("m7", [256, 128], f32r, kind="ExternalInput")
    out = nc.dram_tensor("out", [nb, NOUT], f32, kind="ExternalOutput")

    with tile.TileContext(nc) as tc:
        with (
            tc.tile_pool(name="consts", bufs=1) as consts,
            tc.tile_pool(name="xtp", bufs=2) as xtp,
            tc.tile_pool(name="dp", bufs=2) as dp,
            tc.tile_pool(name="sgp", bufs=1) as sgp,
            tc.tile_pool(name="lnpool", bufs=2) as lnpool,
            tc.tile_pool(name="p7pool", bufs=2) as p7pool,
            tc.tile_pool(name="tree89", bufs=1) as tree89,
            tc.tile_pool(name="p10pool", bufs=2) as p10pool,
            tc.tile_pool(name="outp", bufs=2) as outp,
            tc.tile_pool(name="smalls", bufs=4) as smalls,
            tc.tile_pool(name="zps", bufs=2, space="PSUM") as zps,
            tc.tile_pool(name="p7ps", bufs=2, space="PSUM") as p7ps,
            tc.tile_pool(name="gps", bufs=2, space="PSUM") as gps,
        ):
            wt_sb = consts.tile([128, 8, 1024], f32r)
            nc.sync.dma_start(out=wt_sb, in_=wt.rearrange("(c p) n -> p c n", p=128))
            lp_sb = consts.tile([128, 8, NOUT], f32r)
            nc.sync.dma_start(out=lp_sb, in_=lp.rearrange("(c p) o -> p c o", p=128))
            m7_sb = consts.tile([128, 2, 128], f32r)
            nc.sync.dma_start(out=m7_sb, in_=m7.rearrange("(c p) j -> p c j", p=128))
            ln_eps = consts.tile([128, 1], f32)
            nc.vector.memset(ln_eps, 1e-37)

            for bi in range(n_blocks):
                bs = bi * block
                xt_sb = xtp.tile([128, 8, block], f32r, tag="xt")
                nc.sync.dma_start(
                    out=xt_sb,
                    in_=xt[:, bs : bs + block].rearrange("(c p) b -> p c b", p=128),
                )

                # ---- GEMM1: z[slot, b] accumulated over 8 feature chunks ----
                dcs = {}
                ln_p = ln_n = None
                for nch in range(8):
                    zp = zps.tile([128, block], f32, tag="z")
                    for fc in range(8):
                        nc.tensor.matmul(
                            zp,
                            lhsT=wt_sb[:, fc, nch * 128 : (nch + 1) * 128],
                            rhs=xt_sb[:, fc, :],
                            start=(fc == 0),
                            stop=(fc == 7),
                        )
                    if nch == 0:
                        # levels 0-6 need ln(d) and ln(1-d) of slots 0..127
                        sg_pos = sgp.tile([128, block], f32, tag="sgpos")
                        nc.scalar.activation(out=sg_pos, in_=zp, func=AF.Sigmoid)
                        sg_neg = sgp.tile([128, block], f32, tag="sgneg")
                        nc.scalar.activation(
                            out=sg_neg, in_=zp, func=AF.Sigmoid, scale=-1.0
                        )
                        ln_p = lnpool.tile([128, block], f32r, tag="lnp")
                        nc.scalar.activation(out=ln_p, in_=sg_pos, func=AF.Ln, bias=ln_eps)
                        ln_n = lnpool.tile([128, block], f32r, tag="lnn")
                        nc.scalar.activation(out=ln_n, in_=sg_neg, func=AF.Ln, bias=ln_eps)
                    else:
                        d = dp.tile([128, block], f32r, tag=f"dc{nch}")
                        nc.scalar.activation(out=d, in_=zp, func=AF.Sigmoid)
                        dcs[nch] = d

                # ---- levels 0-6 in log space on the PE ----
                lp7 = p7ps.tile([128, block], f32, tag="logp7")
                nc.tensor.matmul(
                    lp7, lhsT=m7_sb[:, 0, :], rhs=ln_p,
                    start=True, stop=False,
                )
                nc.tensor.matmul(
                    lp7, lhsT=m7_sb[:, 1, :], rhs=ln_n,
                    start=False, stop=True,
                )
                p7 = p7pool.tile([128, block], f32r, tag="p7")
                nc.scalar.activation(out=p7, in_=lp7, func=AF.Exp)

                # ---- levels 7-9, all full-width partition-aligned ----
                p8a = tree89.tile([128, block], f32r, tag="p8a")
                nc.vector.tensor_mul(p8a, p7, dcs[1])
                p8b = tree89.tile([128, block], f32r, tag="p8b")
                nc.vector.tensor_sub(p8b, p7, p8a)

                p9 = []
                t = tree89.tile([128, block], f32r, tag="p9_0")
                nc.vector.tensor_mul(t, p8a, dcs[2])
                p9.append(t)
                t = tree89.tile([128, block], f32r, tag="p9_1")
                nc.vector.tensor_mul(t, p8b, dcs[3])
                p9.append(t)
                t = tree89.tile([128, block], f32r, tag="p9_2")
                nc.vector.tensor_sub(t, p8a, p9[0])
                p9.append(t)
                t = tree89.tile([128, block], f32r, tag="p9_3")
                nc.vector.tensor_sub(t, p8b, p9[1])
                p9.append(t)

                p10 = [None] * 8
                for k in range(4):
                    t = p10pool.tile([128, block], f32r, tag=f"p10_{k}")
                    nc.vector.tensor_mul(t, p9[k], dcs[4 + k])
                    p10[k] = t
                for k in range(4):
                    t = p10pool.tile([128, block], f32r, tag=f"p10_{4 + k}")
                    nc.vector.tensor_sub(t, p9[k], p10[k])
                    p10[4 + k] = t

                # ---- GEMM2 + softmax per 128-row output subtile ----
                for sb in range(block // 128):
                    out_t = outp.tile([128, NOUT], f32, tag="out")
                    sums = smalls.tile([128, 2], f32, tag="sums")
                    for h in range(2):
                        g = gps.tile([128, 512], f32, tag="g")
                        for lc in range(8):
                            nc.tensor.matmul(
                                g,
                                lhsT=p10[lc][:, sb * 128 : (sb + 1) * 128],
                                rhs=lp_sb[:, lc, h * 512 : (h + 1) * 512],
                                start=(lc == 0),
                                stop=(lc == 7),
                            )
                        # |logits| <= max|leaf_pred| (convex combination) so a
                        # max-subtraction pass is unnecessary for fp32 exp.
                        nc.scalar.activation(
                            out=out_t[:, h * 512 : (h + 1) * 512],
                            in_=g,
                            func=AF.Exp,
                            accum_out=sums[:, h : h + 1],
                        )
                    stot = smalls.tile([128, 1], f32, tag="stot")
                    nc.vector.tensor_add(stot, sums[:, 0:1], sums[:, 1:2])
                    rcp = smalls.tile([128, 1], f32, tag="rcp")
                    nc.vector.reciprocal(rcp, stot)
                    nc.vector.tensor_scalar_mul(out_t, out_t, rcp)
                    nc.sync.dma_start(
                        out=out[bs + sb * 128 : bs + (sb + 1) * 128, :], in_=out_t
                    )
    nc.finalize()
    return nc


_PROGRAM_CACHE: dict = {}


def _get_program(n_blocks: int = NBLOCKS, block: int = BLOCK) -> bass.Bass:
    key = (n_blocks, block)
    if key not in _PROGRAM_CACHE:
        _PROGRAM_CACHE[key] = _build_program(n_blocks, block)
    return _PROGRAM_CACHE[key]


def kernel(x, feature_thresholds, leaf_predictions, _trace=False):
    x = np.asarray(x, dtype=np.float32)
    wt, lperm, m7 = _host_prep(feature_thresholds, leaf_predictions)
    xt = np.ascontiguousarray(x.T)  # [F, B]

    nc = _get_program()
    in_maps = []
    for c in range(NCORES):
        shard = np.ascontiguousarray(xt[:, c * BL : (c + 1) * BL])
        in_maps.append({"xt": shard, "wt": wt, "lp": lperm, "m7": m7})

    res = run_bass_kernel_spmd(nc, in_maps, core_ids=list(range(NCORES)), trace=_trace)
    out = np.concatenate([res.results[c]["out"] for c in range(NCORES)], axis=0)
    if _trace:
        return out, res
    return out
